# revision 1
# baseline (speedup 1.0000x reference)
"""SimCLR contrastive-loss kernel for 8 Trainium2 NeuronCores.

Full inputs in, full outputs out.  No collectives: proj_2 is replicated
to every core (host-side slicing/replication only); each core normalizes
all of z2 redundantly and computes its own 1024-row block of the
similarity matrix.  Matmul data is bf16; transposes run on the DMA XBAR
(dma_start_transpose), so the PE does only the 256 main matmuls and the
PSUM is wholly owned by the main loop (4 rotating [128,1024] f32 tiles =
8 banks).  The x side is scaled by -1000/||x|| so PSUM holds -1000*sim
and the DVE reduce_min output is directly the exp bias; ACT exp runs
in-place on PSUM with accumulate.  Exact per-group logsumexp fixup.
Positives are computed exactly in f32 from the raw shards.

Engine budget per column group (8 groups x 8 row-tiles):
  PE   4 matmuls x 512 rows        DVE  reduce_min + y-square reduce
  ACT  exp+accum + y-square accum  GPSIMD  y scale+cast to bf16
"""

import numpy as np

B = 8192          # batch
D = 256           # feature dim
NCORES = 8
R = B // NCORES   # rows per core = 1024
P = 128           # partitions
MT = R // P       # x tiles per core = 8
YT = B // P       # y tiles per core = 64
GROUP = 1024      # columns per logsumexp group
NG = B // GROUP   # groups per row = 8
CHT = 8           # y tiles per chunk (= one group of columns)
NS = 512          # matmul moving free dim (one PSUM bank)
TEMP_INV = 1000.0
LN_TEMP_INV = float(np.log(1000.0))

_CACHE = {}


def _build_nc():
    import concourse.bacc as bacc
    import concourse.mybir as mybir
    from concourse import tile

    f32 = mybir.dt.float32
    bf16 = mybir.dt.bfloat16
    AOT = mybir.AluOpType
    ACT = mybir.ActivationFunctionType
    AXL = mybir.AxisListType

    nc = bacc.Bacc("TRN2", target_bir_lowering=False, debug=False,
                   num_devices=NCORES)

    p1 = nc.dram_tensor("p1", [R, D], f32, kind="ExternalInput")
    p2 = nc.dram_tensor("p2", [B, D], f32, kind="ExternalInput")
    p2s = nc.dram_tensor("p2s", [R, D], f32, kind="ExternalInput")
    res = nc.dram_tensor("res", [P, 2 * MT], f32, kind="ExternalOutput")

    with tile.TileContext(nc) as tc:
        with (
            tc.tile_pool(name="big", bufs=1) as big,
            tc.tile_pool(name="yin", bufs=16) as yin,
            tc.tile_pool(name="scr", bufs=4) as scr,
        ):
            # persistent SBUF tensors
            z2T0 = big.tile([P, B], bf16, tag="z2T0")   # z2^T dims 0..127
            z2T1 = big.tile([P, B], bf16, tag="z2T1")   # z2^T dims 128..255
            xT0 = big.tile([P, R], bf16, tag="xT0")     # (-1000*x^)^T d lo
            xT1 = big.tile([P, R], bf16, tag="xT1")
            xs = big.tile([P, MT * D], f32, tag="xs")   # p1 shard natural
            ys2 = big.tile([P, MT * D], f32, tag="ys2")  # p2 shard natural
            xsb0 = big.tile([P, MT, P], bf16, tag="xsb0")  # scaled x, d lo
            xsb1 = big.tile([P, MT, P], bf16, tag="xsb1")  # scaled x, d hi
            ysb0 = big.tile([P, CHT, P], bf16, tag="ysb0")  # scaled y chunk
            ysb1 = big.tile([P, CHT, P], bf16, tag="ysb1")
            n2x = big.tile([P, MT], f32, tag="n2x")
            n2y = big.tile([P, YT], f32, tag="n2y")
            n2o = big.tile([P, MT], f32, tag="n2o")     # own p2 shard norms
            rix = big.tile([P, MT], f32, tag="rix")     # -1000*rsqrt(n2x)
            riy = big.tile([P, YT], f32, tag="riy")     # rsqrt(n2y)
            rio = big.tile([P, MT], f32, tag="rio")     # rsqrt(n2o)
            tln = big.tile([P, YT], f32, tag="tln")
            praw = big.tile([P, MT], f32, tag="praw")
            qv = big.tile([P, MT], f32, tag="qv")       # -1000*positives
            gmin = big.tile([P, MT * NG], f32, tag="gmin")
            ssum = big.tile([P, MT * NG], f32, tag="ssum")
            t4 = big.tile([P, MT * NG], f32, tag="t4")
            st4 = big.tile([P, MT * NG], f32, tag="st4")
            mrow = big.tile([P, MT], f32, tag="mrow")
            stot = big.tile([P, MT], f32, tag="stot")
            lnst = big.tile([P, MT], f32, tag="lnst")
            outt = big.tile([P, 2 * MT], f32, tag="outt")
            cln1k = big.tile([P, 1], f32, tag="cln1k")
            nc.vector.memset(cln1k[:], LN_TEMP_INV)

            # ---------------- x-side prologue (own p1 shard)
            for m in range(MT):
                nc.sync.dma_start(xs[:, m * D:(m + 1) * D],
                                  p1[m * P:(m + 1) * P, :])
            for m in range(MT):
                sq = scr.tile([P, D], f32, tag="sq")
                nc.scalar.activation(sq[:], xs[:, m * D:(m + 1) * D],
                                     ACT.Square, accum_out=n2x[:, m:m + 1])
            # -1000/sqrt(s) = -exp(-0.5*ln(s) + ln(1000))
            nc.scalar.activation(tln[:, 0:MT], n2x[:], ACT.Ln)
            nc.scalar.activation(rix[:], tln[:, 0:MT], ACT.Exp, scale=-0.5,
                                 bias=cln1k[:])
            nc.vector.tensor_scalar_mul(rix[:], rix[:], -1.0)
            for m in range(MT):
                nc.vector.tensor_scalar(
                    out=xsb0[:, m, :], in0=xs[:, m * D:m * D + P],
                    scalar1=rix[:, m:m + 1], scalar2=None, op0=AOT.mult)
                nc.vector.tensor_scalar(
                    out=xsb1[:, m, :], in0=xs[:, m * D + P:(m + 1) * D],
                    scalar1=rix[:, m:m + 1], scalar2=None, op0=AOT.mult)
                nc.sync.dma_start_transpose(xT0[:, m * P:(m + 1) * P],
                                            xsb0[:, m, :])
                nc.sync.dma_start_transpose(xT1[:, m * P:(m + 1) * P],
                                            xsb1[:, m, :])

            # ---------------- own p2 shard: norms + positives (exact f32)
            for m in range(MT):
                nc.sync.dma_start(ys2[:, m * D:(m + 1) * D],
                                  p2s[m * P:(m + 1) * P, :])
            for m in range(MT):
                sq = scr.tile([P, D], f32, tag="sq")
                nc.scalar.activation(sq[:], ys2[:, m * D:(m + 1) * D],
                                     ACT.Square, accum_out=n2o[:, m:m + 1])
            nc.scalar.activation(tln[:, 0:MT], n2o[:], ACT.Ln)
            nc.scalar.activation(rio[:], tln[:, 0:MT], ACT.Exp, scale=-0.5)
            for m in range(MT):
                sq = scr.tile([P, D], f32, tag="sq")
                nc.gpsimd.tensor_tensor(sq[:], xs[:, m * D:(m + 1) * D],
                                        ys2[:, m * D:(m + 1) * D], AOT.mult)
                nc.vector.reduce_sum(out=praw[:, m:m + 1], in_=sq[:],
                                     axis=AXL.X)
            # q = praw * rix * rio = -1000 * positives
            nc.vector.tensor_mul(qv[:], praw[:], rix[:])
            nc.vector.tensor_mul(qv[:], qv[:], rio[:])

            # ---------------- main: y chunks pipelined with row-block mms
            xTk = (xT0, xT1)
            zTk = (z2T0, z2T1)
            with tc.tile_pool(name="mpsum", bufs=4, space="PSUM") as mpsum:
                for g in range(NG):
                    # y-chunk prep: 8 tiles -> normalized bf16 -> z2T cols
                    c0 = g * CHT
                    ytiles = []
                    for t in range(CHT):
                        yt = yin.tile([P, D], f32, tag="yt")
                        nc.sync.dma_start(
                            yt[:], p2[(c0 + t) * P:(c0 + t + 1) * P, :])
                        ytiles.append(yt)
                        # square+sum: gpsimd multiply, DVE free-axis reduce
                        sq = scr.tile([P, D], f32, tag="sq")
                        nc.gpsimd.tensor_tensor(sq[:], yt[:], yt[:], AOT.mult)
                        nc.vector.reduce_sum(
                            out=n2y[:, c0 + t:c0 + t + 1], in_=sq[:],
                            axis=AXL.X)
                    nc.scalar.activation(tln[:, c0:c0 + CHT],
                                         n2y[:, c0:c0 + CHT], ACT.Ln)
                    nc.scalar.activation(riy[:, c0:c0 + CHT],
                                         tln[:, c0:c0 + CHT],
                                         ACT.Exp, scale=-0.5)
                    for t in range(CHT):
                        yt = ytiles[t]
                        nc.gpsimd.tensor_scalar(
                            out=ysb0[:, t, :], in0=yt[:, 0:P],
                            scalar1=riy[:, c0 + t:c0 + t + 1],
                            scalar2=None, op0=AOT.mult)
                        nc.gpsimd.tensor_scalar(
                            out=ysb1[:, t, :], in0=yt[:, P:D],
                            scalar1=riy[:, c0 + t:c0 + t + 1],
                            scalar2=None, op0=AOT.mult)
                        nc.sync.dma_start_transpose(
                            z2T0[:, (c0 + t) * P:(c0 + t + 1) * P],
                            ysb0[:, t, :])
                        nc.sync.dma_start_transpose(
                            z2T1[:, (c0 + t) * P:(c0 + t + 1) * P],
                            ysb1[:, t, :])

                    # row-block matmuls + logsumexp pieces for this col group
                    for m in range(MT):
                        col = m * NG + g
                        pg = mpsum.tile([P, GROUP], f32, tag="pg")
                        for n in range(GROUP // NS):
                            for k in range(2):
                                nc.tensor.matmul(
                                    pg[:, n * NS:(n + 1) * NS],
                                    xTk[k][:, m * P:(m + 1) * P],
                                    zTk[k][:, g * GROUP + n * NS:
                                           g * GROUP + (n + 1) * NS],
                                    start=(k == 0), stop=(k == 1))
                        nc.vector.tensor_reduce(
                            out=gmin[:, col:col + 1], in_=pg[:],
                            axis=AXL.X, op=AOT.min)
                        # exp(1000*s - 1000*max) in place on PSUM, row-sums
                        nc.scalar.activation(pg[:], pg[:], ACT.Exp,
                                             scale=-1.0,
                                             bias=gmin[:, col:col + 1],
                                             accum_out=ssum[:, col:col + 1])

            # ---------------- exact fixup across groups, outputs
            for m in range(MT):
                c0, c1 = m * NG, (m + 1) * NG
                nc.vector.tensor_reduce(out=mrow[:, m:m + 1],
                                        in_=gmin[:, c0:c1],
                                        axis=AXL.X, op=AOT.min)
                nc.scalar.activation(t4[:, c0:c1], gmin[:, c0:c1],
                                     ACT.Exp, scale=-1.0,
                                     bias=mrow[:, m:m + 1])
                nc.vector.tensor_mul(st4[:, c0:c1], t4[:, c0:c1],
                                     ssum[:, c0:c1])
                nc.vector.reduce_sum(out=stot[:, m:m + 1], in_=st4[:, c0:c1],
                                     axis=AXL.X)
            nc.scalar.activation(lnst[:], stot[:], ACT.Ln)
            # loss rows = ln(stot) - mrow + q ;  q column = -1000*pos
            nc.vector.tensor_sub(outt[:, 0:MT], lnst[:], mrow[:])
            nc.vector.tensor_add(outt[:, 0:MT], outt[:, 0:MT], qv[:])
            nc.vector.tensor_copy(outt[:, MT:2 * MT], qv[:])
            nc.sync.dma_start(res[:, :], outt[:])

    nc.compile()
    return nc


def _get_nc():
    if "nc" not in _CACHE:
        _CACHE["nc"] = _build_nc()
    return _CACHE["nc"]


def run_cores(proj_1, proj_2, **spmd_kwargs):
    """Run the SPMD kernel; returns BassKernelResults."""
    from concourse.bass_utils import run_bass_kernel_spmd

    p1 = np.ascontiguousarray(np.asarray(proj_1, dtype=np.float32))
    p2 = np.ascontiguousarray(np.asarray(proj_2, dtype=np.float32))
    assert p1.shape == (B, D) and p2.shape == (B, D)
    in_maps = [
        {"p1": p1[c * R:(c + 1) * R], "p2": p2,
         "p2s": p2[c * R:(c + 1) * R]}
        for c in range(NCORES)
    ]
    nc = _get_nc()
    br = run_bass_kernel_spmd(nc, in_maps, core_ids=list(range(NCORES)),
                              **spmd_kwargs)
    return br


def kernel(proj_1, proj_2):
    br = run_cores(proj_1, proj_2)
    loss_sum = np.float64(0.0)
    q_sum = np.float64(0.0)
    for r in br.results:
        out = r["res"]
        loss_sum += np.float64(out[:, :MT].astype(np.float64).sum())
        q_sum += np.float64(out[:, MT:].astype(np.float64).sum())
    loss = np.float32(loss_sum / B)
    pos = np.float32(-q_sum / TEMP_INV)
    return (loss, pos)



# revision 13
# speedup vs baseline: 2.5591x; 2.5591x over previous
"""SimCLR contrastive-loss kernel for 8 Trainium2 NeuronCores (v2).

Full inputs in, full outputs out.  proj_2 is host-cast to bf16 and
replicated to every core; each core computes its 1024-row block of the
similarity matrix against all 8192 columns.

Key structure (vs v1 baseline at 429us):
- y (proj_2) staged as bf16: halves HBM traffic, enables DVE 2x modes.
- All transposes on the DMA XBAR with multi-tile 3D-output APs:
  one dma_start_transpose per (group, dim-half) moves [128, 8x128]
  (out[p, t, j] = in[j, t*128+p]), so 18 transpose dispatches total
  instead of 144.
- No GPSIMD tensor_scalar (measured 2us per [128,128]); y scaling runs
  on DVE in 2x_1p mode (all-bf16 operands, f32 [P,1] scalars exempt).
- rsqrt via bitcast+Newton entirely on DVE; no Ln anywhere on device
  (final log runs on host), so ACT uses only {Square, Exp} = one
  activation-table set -> a single ACT_TABLE_LOAD.
- Row-max (min of negated psum) reduces split DVE/Pool per row-tile.
- PSUM: 4 rotating [128,1024] f32 tiles = all 8 banks; k-outer matmul
  order halves LDWEIGHTS.

Per-group logsumexp is exact: per-group min & sum, then an exact
cross-group fixup; positives are computed exactly in f32 from the raw
f32 shards.
"""

import numpy as np

B = 8192          # batch
D = 256           # feature dim
NCORES = 8
R = B // NCORES   # rows per core = 1024
P = 128           # partitions
MT = R // P       # x row-tiles per core = 8
NT = B // P       # y tiles = 64
GROUP = 1024      # columns per logsumexp group
NG = B // GROUP   # groups = 8
CHT = 8           # y tiles per group
TEMP_INV = 1000.0
MAGIC = 0x5F3759DF
N_DVE_MIN = 3     # row-tiles per group whose min-reduce runs on DVE

_CACHE = {}


def _build_nc():
    import concourse.bacc as bacc
    import concourse.mybir as mybir
    from concourse import tile

    f32 = mybir.dt.float32
    bf16 = mybir.dt.bfloat16
    i32 = mybir.dt.int32
    AOT = mybir.AluOpType
    ACT = mybir.ActivationFunctionType
    AXL = mybir.AxisListType

    nc = bacc.Bacc("TRN2", target_bir_lowering=False, debug=False,
                   num_devices=NCORES)

    p1 = nc.dram_tensor("p1", [R, D], f32, kind="ExternalInput")
    p2b = nc.dram_tensor("p2b", [B, D], bf16, kind="ExternalInput")
    p2s = nc.dram_tensor("p2s", [R, D], f32, kind="ExternalInput")
    res = nc.dram_tensor("res", [P, 3 * MT], f32, kind="ExternalOutput")

    p1r = p1.reshape([MT, P, D])
    p2r = p2b.reshape([NT, P, D])
    p2sr = p2s.reshape([MT, P, D])

    with tile.TileContext(nc) as tc:
        with (
            tc.tile_pool(name="big", bufs=1) as big,
            tc.tile_pool(name="ysb", bufs=4) as ysbp,
            tc.tile_pool(name="scr", bufs=4) as scr,
        ):
            # persistent SBUF tensors
            ys = big.tile([P, NT, D], bf16, tag="ys")      # all of y, bf16
            xs = big.tile([P, MT, D], f32, tag="xs")       # own p1 shard
            ys2 = big.tile([P, MT, D], f32, tag="ys2")     # own p2 shard
            xsb0 = big.tile([P, MT, P], bf16, tag="xsb0")  # scaled x, d lo
            xsb1 = big.tile([P, MT, P], bf16, tag="xsb1")  # scaled x, d hi
            xT0 = big.tile([P, MT, P], bf16, tag="xT0")    # x^T d lo
            xT1 = big.tile([P, MT, P], bf16, tag="xT1")    # x^T d hi
            z2T0 = big.tile([P, NT, P], bf16, tag="z2T0")  # y^T d lo
            z2T1 = big.tile([P, NT, P], bf16, tag="z2T1")  # y^T d hi
            n2x = big.tile([P, MT], f32, tag="n2x")
            n2o = big.tile([P, MT], f32, tag="n2o")
            n2y = big.tile([P, NT], f32, tag="n2y")
            rix = big.tile([P, MT], f32, tag="rix")        # -1000*rsqrt(n2x)
            rio = big.tile([P, MT], f32, tag="rio")        # rsqrt(n2o)
            riy = big.tile([P, NT], f32, tag="riy")        # rsqrt(n2y)
            praw = big.tile([P, MT], f32, tag="praw")
            qv = big.tile([P, MT], f32, tag="qv")          # -1000*positives
            gmin = big.tile([P, MT * NG], f32, tag="gmin")
            ssum = big.tile([P, MT * NG], f32, tag="ssum")
            t4 = big.tile([P, MT * NG], f32, tag="t4")
            st4 = big.tile([P, MT * NG], f32, tag="st4")
            mrow = big.tile([P, MT], f32, tag="mrow")
            stot = big.tile([P, MT], f32, tag="stot")
            outt = big.tile([P, 3 * MT], f32, tag="outt")
            qmag = big.tile([P, NT], i32, tag="qmag")
            qa = big.tile([P, NT], f32, tag="qa")
            qb = big.tile([P, NT], f32, tag="qb")
            nc.vector.memset(qmag[:], MAGIC)

            def quake_rsqrt(dst, src, c0, c1, final_scale=None, tt=None):
                """dst[:, c0:c1] = rsqrt(src[:, c0:c1]).

                tensor_scalar steps stay on DVE; tensor_tensor steps go to
                `tt` (default DVE; pass nc.gpsimd to offload to Pool).
                """
                # NOTE: keep every op (and both bitcast endpoints) on DVE:
                # same-engine program order guarantees correctness even if
                # bitcast APs are invisible to cross-engine dep tracking.
                tt = nc.vector
                d = dst[:, c0:c1]
                s = src[:, c0:c1]
                nc.vector.tensor_scalar(
                    out=d.bitcast(i32), in0=s.bitcast(i32), scalar1=1,
                    scalar2=None, op0=AOT.arith_shift_right)
                tt.tensor_tensor(
                    out=d.bitcast(i32), in0=qmag[:, c0:c1],
                    in1=d.bitcast(i32), op=AOT.subtract)
                for _ in range(2):
                    tt.tensor_tensor(out=qa[:, c0:c1], in0=d, in1=d,
                                     op=AOT.mult)
                    tt.tensor_tensor(out=qa[:, c0:c1],
                                     in0=qa[:, c0:c1], in1=s,
                                     op=AOT.mult)
                    nc.vector.tensor_scalar(out=qb[:, c0:c1],
                                            in0=qa[:, c0:c1], scalar1=-0.5,
                                            scalar2=1.5, op0=AOT.mult,
                                            op1=AOT.add)
                    tt.tensor_tensor(out=d, in0=d, in1=qb[:, c0:c1],
                                     op=AOT.mult)
                if final_scale is not None:
                    nc.vector.tensor_scalar_mul(d, d, final_scale)

            # ---------------- DMA: x shard, own-y shard, then all of y
            nc.sync.dma_start(xs[:, :, :], p1r[:, :, :].transpose([1, 0, 2]))
            nc.sync.dma_start(ys2[:, :, :], p2sr[:, :, :].transpose([1, 0, 2]))
            for g in range(NG):
                nc.sync.dma_start(
                    ys[:, g * CHT:(g + 1) * CHT, :],
                    p2r[g * CHT:(g + 1) * CHT, :, :].transpose([1, 0, 2]))

            # ---------------- x-side prologue
            # squares via Pool mult + DVE reduce so the n2x -> quake
            # (bitcast read) edge stays within DVE program order
            for m in range(MT):
                sq = scr.tile([P, D], f32, tag="sq")
                nc.gpsimd.tensor_tensor(out=sq[:], in0=xs[:, m, :],
                                        in1=xs[:, m, :], op=AOT.mult)
                nc.vector.reduce_sum(out=n2x[:, m:m + 1], in_=sq[:],
                                     axis=AXL.X)
            quake_rsqrt(rix, n2x, 0, MT, final_scale=-TEMP_INV)
            for m in range(MT):
                nc.vector.tensor_scalar(
                    out=xsb0[:, m, :], in0=xs[:, m, 0:P],
                    scalar1=rix[:, m:m + 1], scalar2=None, op0=AOT.mult)
                nc.vector.tensor_scalar(
                    out=xsb1[:, m, :], in0=xs[:, m, P:D],
                    scalar1=rix[:, m:m + 1], scalar2=None, op0=AOT.mult)
            nc.sync.dma_start_transpose(xT0[:, :, :], xsb0[:].opt())
            nc.sync.dma_start_transpose(xT1[:, :, :], xsb1[:].opt())

            # ---------------- own p2 shard: norms + exact positives
            for m in range(MT):
                sq = scr.tile([P, D], f32, tag="sq")
                nc.gpsimd.tensor_tensor(out=sq[:], in0=ys2[:, m, :],
                                        in1=ys2[:, m, :], op=AOT.mult)
                nc.vector.reduce_sum(out=n2o[:, m:m + 1], in_=sq[:],
                                     axis=AXL.X)
            quake_rsqrt(rio, n2o, 0, MT)
            for m in range(MT):
                pm = scr.tile([P, D], f32, tag="pm")
                nc.gpsimd.tensor_tensor(pm[:], xs[:, m, :], ys2[:, m, :],
                                        AOT.mult)
                nc.vector.reduce_sum(out=praw[:, m:m + 1], in_=pm[:],
                                     axis=AXL.X)
            nc.vector.tensor_mul(qv[:], praw[:], rix[:])
            nc.vector.tensor_mul(qv[:], qv[:], rio[:])

            # ---------------- main pipelined loop
            with tc.tile_pool(name="mpsum", bufs=4, space="PSUM") as mpsum:
                for i in range(NG + 2):
                    # stage S: squares for group i (mult on Pool, sum on DVE)
                    if i < NG:
                        for t in range(CHT):
                            c = i * CHT + t
                            sqb = scr.tile([P, D], bf16, tag="sqb")
                            nc.gpsimd.tensor_tensor(
                                out=sqb[:], in0=ys[:, c, :], in1=ys[:, c, :],
                                op=AOT.mult)
                            nc.vector.reduce_sum(out=n2y[:, c:c + 1],
                                                 in_=sqb[:], axis=AXL.X)
                    # stage Q: riy for wave {i-1, i}
                    if i in (1, 3, 5, 7):
                        quake_rsqrt(riy, n2y, (i - 1) * CHT, (i + 1) * CHT)
                    # stage C: scale + transpose for group i-1
                    if 0 <= i - 1 < NG:
                        g = i - 1
                        ysb0 = ysbp.tile([P, CHT, P], bf16, tag="ysb0")
                        ysb1 = ysbp.tile([P, CHT, P], bf16, tag="ysb1")
                        for t in range(CHT):
                            c = g * CHT + t
                            nc.vector.tensor_scalar(
                                out=ysb0[:, t, :], in0=ys[:, c, 0:P],
                                scalar1=riy[:, c:c + 1], scalar2=None,
                                op0=AOT.mult)
                            # d-hi half scaled on ACT (Copy is in the same
                            # act-table set as Exp -> no table reload)
                            nc.scalar.activation(
                                ysb1[:, t, :], ys[:, c, P:D], ACT.Copy,
                                scale=riy[:, c:c + 1])
                        nc.sync.dma_start_transpose(
                            z2T0[:, g * CHT:(g + 1) * CHT, :], ysb0[:].opt())
                        nc.sync.dma_start_transpose(
                            z2T1[:, g * CHT:(g + 1) * CHT, :], ysb1[:].opt())
                    # stage M: matmuls + min + exp for group i-2
                    if 0 <= i - 2 < NG:
                        g = i - 2
                        ga, gb = g * CHT, g * CHT + CHT // 2
                        for m in range(MT):
                            col = m * NG + g
                            pg = mpsum.tile([P, GROUP], f32, tag="pg")
                            nc.tensor.matmul(
                                pg[:, 0:GROUP // 2], xT0[:, m, :],
                                z2T0[:, ga:gb, :], start=True, stop=False)
                            nc.tensor.matmul(
                                pg[:, GROUP // 2:GROUP], xT0[:, m, :],
                                z2T0[:, gb:gb + CHT // 2, :],
                                start=True, stop=False)
                            nc.tensor.matmul(
                                pg[:, 0:GROUP // 2], xT1[:, m, :],
                                z2T1[:, ga:gb, :], start=False, stop=True)
                            nc.tensor.matmul(
                                pg[:, GROUP // 2:GROUP], xT1[:, m, :],
                                z2T1[:, gb:gb + CHT // 2, :],
                                start=False, stop=True)
                            nc.vector.tensor_reduce(
                                out=gmin[:, col:col + 1], in_=pg[:],
                                axis=AXL.X, op=AOT.min)
                            nc.scalar.activation(
                                pg[:], pg[:], ACT.Exp, scale=-1.0,
                                bias=gmin[:, col:col + 1],
                                accum_out=ssum[:, col:col + 1])

            # ---------------- exact cross-group fixup (log happens on host)
            for m in range(MT):
                c0, c1 = m * NG, (m + 1) * NG
                nc.vector.tensor_reduce(out=mrow[:, m:m + 1],
                                        in_=gmin[:, c0:c1],
                                        axis=AXL.X, op=AOT.min)
                nc.scalar.activation(t4[:, c0:c1], gmin[:, c0:c1],
                                     ACT.Exp, scale=-1.0,
                                     bias=mrow[:, m:m + 1])
                nc.gpsimd.tensor_tensor(out=st4[:, c0:c1], in0=t4[:, c0:c1],
                                        in1=ssum[:, c0:c1], op=AOT.mult)
                nc.vector.reduce_sum(out=stot[:, m:m + 1], in_=st4[:, c0:c1],
                                     axis=AXL.X)
            nc.vector.tensor_copy(outt[:, 0:MT], stot[:])
            nc.vector.tensor_copy(outt[:, MT:2 * MT], mrow[:])
            nc.vector.tensor_copy(outt[:, 2 * MT:3 * MT], qv[:])
            nc.sync.dma_start(res[:, :], outt[:])

    nc.compile()
    return nc


def _get_nc():
    if "nc" not in _CACHE:
        _CACHE["nc"] = _build_nc()
    return _CACHE["nc"]


def run_cores(proj_1, proj_2, **spmd_kwargs):
    """Run the SPMD kernel; returns BassKernelResults."""
    import ml_dtypes
    from concourse.bass_utils import run_bass_kernel_spmd

    p1 = np.ascontiguousarray(np.asarray(proj_1, dtype=np.float32))
    p2 = np.ascontiguousarray(np.asarray(proj_2, dtype=np.float32))
    assert p1.shape == (B, D) and p2.shape == (B, D)
    p2bf = np.ascontiguousarray(p2.astype(ml_dtypes.bfloat16))
    in_maps = [
        {"p1": p1[c * R:(c + 1) * R], "p2b": p2bf,
         "p2s": p2[c * R:(c + 1) * R]}
        for c in range(NCORES)
    ]
    nc = _get_nc()
    br = run_bass_kernel_spmd(nc, in_maps, core_ids=list(range(NCORES)),
                              **spmd_kwargs)
    return br


def kernel(proj_1, proj_2):
    br = run_cores(proj_1, proj_2)
    loss_sum = np.float64(0.0)
    q_sum = np.float64(0.0)
    for r in br.results:
        out = np.asarray(r["res"], dtype=np.float64)
        stot = out[:, 0:MT]
        mrow = out[:, MT:2 * MT]
        qv = out[:, 2 * MT:3 * MT]
        loss_sum += float((np.log(stot) - mrow + qv).sum())
        q_sum += float(qv.sum())
    loss = np.float32(loss_sum / B)
    pos = np.float32(-q_sum / TEMP_INV)
    return (loss, pos)
